# revision 1
# baseline (speedup 1.0000x reference)
"""Trainium2 Bass kernel for a latent ConvCNP (gaussian encoder -> CNN ->
latent samples -> gaussian interpolator), data-parallel over batch on 8
NeuronCores.

Contract: kernel(**inputs) takes the full unsharded inputs (numpy) and
returns the full (NS, nb, ntar, 2C) output.
"""

import sys

sys.path.insert(0, "/opt/trn_rl_repo")

import math

import numpy as np

import concourse.bacc as bacc
import concourse.mybir as mybir
import concourse.tile as tile
from concourse import bass_utils
from concourse.tile_rust import add_dep_helper

F32 = mybir.dt.float32
F32R = mybir.dt.float32r
AF = mybir.ActivationFunctionType
ALU = mybir.AluOpType

# problem constants (fixed by the reference problem)
EPS = 1e-6
C = 3
NBASIS = 5
NS = 4
RIN = 16
ROUT = 32
KW = 5
NB = 16          # full batch
NPTS = 2048
NTAR = 256
NCORES = 8
NBL = NB // NCORES   # batches per core
NCH = NPTS // 128    # 16 point-chunks per (b, c)
KAPPA = math.sqrt(math.pi) / 2.0  # exp(-x^2) = KAPPA * Derivative_Erf(x)

_CACHE = {}


def _build(m, loop_r=1):
    """Build the per-core Bass program. m = grid size (312)."""
    mts = [128] * (m // 128) + ([m % 128] if m % 128 else [])
    njt = len(mts)
    mp = m + 4  # padded conv width

    nc = bacc.Bacc("TRN2", target_bir_lowering=False, debug=False)

    # ---- per-core DRAM inputs ----
    d_xr = nc.dram_tensor("xr", [NBL, 128, NCH * C], F32, kind="ExternalInput")
    d_yr = nc.dram_tensor("yr", [NBL, 128, 2 * NCH * C], F32, kind="ExternalInput")
    d_xtr = nc.dram_tensor("xtr", [NBL, 128, C * NTAR], F32, kind="ExternalInput")
    d_grep = nc.dram_tensor("grep", [128, m], F32, kind="ExternalInput")
    d_bj = nc.dram_tensor("bj", [128, njt], F32, kind="ExternalInput")
    d_gw = nc.dram_tensor("gw", [2 * C, RIN], F32, kind="ExternalInput")
    d_gbn = nc.dram_tensor("gbn", [RIN, 1], F32, kind="ExternalInput")
    d_w1 = nc.dram_tensor("w1t", [RIN, KW * ROUT], F32, kind="ExternalInput")
    d_w2 = nc.dram_tensor("w2t", [ROUT, KW * ROUT], F32, kind="ExternalInput")
    d_w3 = nc.dram_tensor("w3t", [ROUT, KW * ROUT], F32, kind="ExternalInput")
    d_b123 = nc.dram_tensor("b123", [ROUT, 3], F32, kind="ExternalInput")
    d_linw = nc.dram_tensor("linw", [ROUT, 2 * C * NBASIS], F32, kind="ExternalInput")
    d_linbr = nc.dram_tensor("linbr", [128, 2 * C * NBASIS], F32, kind="ExternalInput")
    d_epsb = nc.dram_tensor("epsb", [NBL, 128, NBASIS * C * NS], F32, kind="ExternalInput")
    d_lowb = nc.dram_tensor("lowb", [128, C * NS * 2 * C * NBASIS], F32, kind="ExternalInput")
    d_lobb = nc.dram_tensor("lobb", [128, NS * 2 * C], F32, kind="ExternalInput")
    d_alpha = nc.dram_tensor("alphas", [1, 4], F32, kind="ExternalInput")  # unused on dev
    d_out = nc.dram_tensor("out", [NS, NBL, NTAR, 2 * C], F32, kind="ExternalOutput")

    # scale constants are baked at build time via alphas passed in _CACHE key;
    # instead we pass them as python floats through build closure -> see build()
    alpha_enc = _build.alpha_enc  # (3,) python floats
    alpha_int = _build.alpha_int  # float
    epsp = EPS / KAPPA

    with tile.TileContext(nc) as tc:
        import contextlib

        est = contextlib.ExitStack()
        with est:
            p_cst = est.enter_context(tc.tile_pool(name="cst", bufs=1))
            p_io = est.enter_context(tc.tile_pool(name="io", bufs=1))
            p_act = est.enter_context(tc.tile_pool(name="eact", bufs=3))
            p_ei = est.enter_context(tc.tile_pool(name="ei", bufs=2 * njt))
            p_feat = est.enter_context(tc.tile_pool(name="feat", bufs=2))
            p_hc = est.enter_context(tc.tile_pool(name="hc", bufs=2))
            p_sm = est.enter_context(tc.tile_pool(name="sm", bufs=3))
            p_z = est.enter_context(tc.tile_pool(name="z", bufs=3))
            p_zz2 = est.enter_context(tc.tile_pool(name="zz2", bufs=njt + 1))
            p_ot = est.enter_context(tc.tile_pool(name="ot", bufs=2))
            ps_e = est.enter_context(tc.tile_pool(name="pse", bufs=2, space="PSUM"))
            ps_c = est.enter_context(tc.tile_pool(name="psc", bufs=2, space="PSUM"))
            ps_h = est.enter_context(tc.tile_pool(name="psh", bufs=2, space="PSUM"))
            ps_o = est.enter_context(tc.tile_pool(name="pso", bufs=2, space="PSUM"))

            # ---- constant loads (outside any bench loop) ----
            grep = p_cst.tile([128, m], F32)
            bj = p_cst.tile([128, njt], F32)
            gw = p_cst.tile([2 * C, RIN], F32R)
            gbn = p_cst.tile([RIN, 1], F32)
            w1 = p_cst.tile([RIN, KW * ROUT], F32R)
            w2 = p_cst.tile([ROUT, KW * ROUT], F32R)
            w3 = p_cst.tile([ROUT, KW * ROUT], F32R)
            b123 = p_cst.tile([ROUT, 3], F32)
            linw = p_cst.tile([ROUT, 2 * C * NBASIS], F32R)
            linbr = p_cst.tile([128, 2 * C * NBASIS], F32)
            lowb = p_cst.tile([128, C * NS * 2 * C * NBASIS], F32)
            lobb = p_cst.tile([128, NS * 2 * C], F32)
            nc.sync.dma_start(grep[:], d_grep.ap())
            nc.sync.dma_start(bj[:], d_bj.ap())
            nc.sync.dma_start(gw[:], d_gw.ap().bitcast(F32R))
            nc.sync.dma_start(gbn[:], d_gbn.ap())
            nc.sync.dma_start(w1[:], d_w1.ap().bitcast(F32R))
            nc.sync.dma_start(w2[:], d_w2.ap().bitcast(F32R))
            nc.sync.dma_start(w3[:], d_w3.ap().bitcast(F32R))
            nc.sync.dma_start(b123[:], d_b123.ap())
            nc.sync.dma_start(linw[:], d_linw.ap().bitcast(F32R))
            nc.sync.dma_start(linbr[:], d_linbr.ap())
            nc.sync.dma_start(lowb[:], d_lowb.ap())
            nc.sync.dma_start(lobb[:], d_lobb.ap())

            def body(_=None):
                # ---- per-batch loads ----
                xrs, yrs, xtrs, epss = [], [], [], []
                for b in range(NBL):
                    xr = p_io.tile([128, NCH * C], F32, tag="xr")
                    yr = p_io.tile([128, 2 * NCH * C], F32R, tag="yr")
                    xtr = p_io.tile([128, C * NTAR], F32, tag="xtr")
                    epsb = p_io.tile([128, NBASIS * C * NS], F32, tag="epsb")
                    nc.sync.dma_start(xr[:], d_xr.ap()[b])
                    nc.sync.dma_start(yr[:], d_yr.ap()[b].bitcast(F32R))
                    nc.sync.dma_start(xtr[:], d_xtr.ap()[b])
                    nc.sync.dma_start(epsb[:], d_epsb.ap()[b])
                    xrs.append(xr); yrs.append(yr); xtrs.append(xtr)
                    epss.append(epsb)

                # ---- phase A: erf_derivative table ----
                # d[p,(ch,j)] = g_j - x[b, ch*128+p, c] on DVE (one op), then
                # one big ACT per (b,c) amortizes the ~350cyc ACT overhead.
                enc_last_act = [None, None]
                feats = []
                for b in range(NBL):
                    f64 = p_feat.tile([35, m], F32, tag="f64")
                    for c in range(C):
                        d6 = p_act.tile([128, NCH * m], F32, tag="d6")
                        gv = grep[:].unsqueeze(1).broadcast_to([128, NCH, m])
                        xv = (
                            xrs[b][:]
                            .rearrange("p (ch c) -> p ch c", ch=NCH, c=C)[:, :, c : c + 1]
                            .broadcast_to([128, NCH, m])
                        )
                        nc.vector.tensor_tensor(
                            d6[:].rearrange("p (ch j) -> p ch j", ch=NCH, j=m),
                            gv, xv, op=ALU.subtract,
                        )
                        E6 = p_act.tile([128, NCH * m], F32R, tag="E6")
                        ai = nc.scalar.activation(
                            E6[:], d6[:], AF.Derivative_Erf,
                            scale=float(alpha_enc[c]),
                        )
                        enc_last_act[b] = ai
                        psum = ps_e.tile([2, m], F32)
                        for ch in range(NCH):
                            idx = 2 * (ch * C + c)
                            nc.tensor.matmul(
                                psum[:], yrs[b][:, idx : idx + 2],
                                E6[:, ch * m : (ch + 1) * m],
                                start=(ch == 0), stop=(ch == NCH - 1),
                            )
                        hh = p_sm.tile([2, m], F32, tag="hh")
                        nc.vector.tensor_copy(hh[:], psum[:])
                        # h0 -> f64[c], h1 -> f64[32+c] (two contiguous DMAs;
                        # a strided-partition write confuses tile dep tracking)
                        nc.sync.dma_start(f64[c : c + 1], hh[0:1])
                        nc.sync.dma_start(f64[32 + c : 33 + c], hh[1:2])
                    # n_h1 = h1 / (h0 + eps/kappa)
                    tmp3 = p_sm.tile([3, m], F32, tag="t3")
                    nc.vector.tensor_scalar_add(tmp3[:], f64[0:3], float(epsp))
                    rec = p_sm.tile([35, m], F32, tag="rec")
                    scr = p_sm.tile([35, m], F32, tag="scr")
                    nc.vector.reciprocal_approx_accurate(rec[32:35], tmp3[:], scr[32:35])
                    nc.vector.tensor_tensor(f64[32:35], f64[32:35], rec[32:35], op=ALU.mult)
                    featp = p_feat.tile([2 * C, m], F32R, tag="featp")
                    nc.sync.dma_start(featp[0:3], f64[0:3].bitcast(F32R))
                    nc.sync.dma_start(featp[3:6], f64[32:35].bitcast(F32R))
                    feats.append(featp)

                # interp gaussians (same table), chained to keep engine order
                eis = []
                prev = None
                for b in range(NBL):
                    ei_b = []
                    for jt in range(njt):
                        jts = mts[jt]
                        ei = p_ei.tile([128, C * NTAR], F32, tag="ei")
                        # bias (-g_j * alpha_int) is channel-independent: one
                        # ACT covers all 3 channels (free = 768)
                        ai = nc.scalar.activation(
                            ei[:jts], xtrs[b][:jts], AF.Derivative_Erf,
                            bias=bj[:jts, jt : jt + 1],
                            scale=float(alpha_int),
                        )
                        if prev is None:
                            add_dep_helper(ai.ins, enc_last_act[0].ins, sync=False)
                            add_dep_helper(ai.ins, enc_last_act[1].ins, sync=False)
                        else:
                            add_dep_helper(ai.ins, prev.ins, sync=False)
                        prev = ai
                        ei_b.append(ei)
                    eis.append(ei_b)
                ei_last = prev

                # ---- phase B: natural_log_exp table ----
                for b in range(NBL):
                    # rep = gw.T @ feat ; sigma = 1/(1+exp(-rep-gb))
                    rep_ps = ps_c.tile([RIN, m], F32, tag="cps")
                    nc.tensor.matmul(rep_ps[:], gw[:], feats[b][:], start=True, stop=True)
                    e1 = p_sm.tile([RIN, m], F32, tag="e1")
                    ai = nc.scalar.activation(
                        e1[:], rep_ps[:], AF.Exp, bias=gbn[:], scale=-1.0
                    )
                    add_dep_helper(ai.ins, ei_last.ins, sync=False)
                    d1 = p_sm.tile([RIN, m], F32, tag="d1")
                    nc.vector.tensor_scalar_add(d1[:], e1[:], 1.0)
                    sg1 = p_sm.tile([RIN, m], F32, tag="sg1")
                    scr1 = p_sm.tile([RIN, m], F32, tag="scr1")
                    nc.vector.reciprocal_approx_accurate(sg1[:], d1[:], scr1[:])
                    h0c = p_hc.tile([RIN, mp], F32R, tag="h0c")
                    nc.vector.tensor_copy(h0c[:, 2 : 2 + m], sg1[:])  # fp32->fp32r

                    # zero pad columns via ACT Identity(scale=0)
                    nc.scalar.activation(
                        h0c[:RIN, 0:2], grep[:RIN, 0:2], AF.Identity, scale=0.0
                    )
                    nc.scalar.activation(
                        h0c[:RIN, 2 + m : mp], grep[:RIN, 0:2], AF.Identity, scale=0.0
                    )

                    # conv stack (5-tap, SAME) as 5 shifted accumulating matmuls
                    hin = h0c
                    houts = []
                    for li, (wt, cin) in enumerate([(w1, RIN), (w2, ROUT), (w3, ROUT)]):
                        cps = ps_c.tile([ROUT, m], F32, tag="cps")
                        for dk in range(KW):
                            nc.tensor.matmul(
                                cps[:], wt[:cin, dk * ROUT : (dk + 1) * ROUT],
                                hin[:cin, dk : dk + m],
                                start=(dk == 0), stop=(dk == KW - 1),
                            )
                        if li < 2:
                            hout = p_hc.tile([ROUT, mp], F32R, tag=f"h{li + 1}c")
                            nc.scalar.activation(
                                hout[:, 2 : 2 + m], cps[:], AF.Relu,
                                bias=b123[:, li : li + 1], scale=1.0,
                            )
                            nc.scalar.activation(
                                hout[:, 0:2], grep[:ROUT, 0:2], AF.Identity, scale=0.0
                            )
                            nc.scalar.activation(
                                hout[:, 2 + m : mp], grep[:ROUT, 0:2], AF.Identity,
                                scale=0.0,
                            )
                        else:
                            hout = p_hc.tile([ROUT, m], F32R, tag="h3c")
                            nc.scalar.activation(
                                hout[:], cps[:], AF.Identity,
                                bias=b123[:, 2:3], scale=1.0,
                            )
                        houts.append(hout)
                        hin = hout
                    h3 = houts[2]

                    # h_grid -> z -> zz2 per grid tile
                    zz2s = []
                    for jt in range(njt):
                        jts = mts[jt]
                        j0 = jt * 128
                        hg = ps_h.tile([128, 2 * C * NBASIS], F32, tag="hg")
                        nc.tensor.matmul(
                            hg[:jts], h3[:, j0 : j0 + jts], linw[:],
                            start=True, stop=True,
                        )
                        hgs = p_sm.tile([128, 2 * C * NBASIS], F32, tag="hgs")
                        nc.vector.tensor_tensor(
                            hgs[:jts], hg[:jts], linbr[:jts], op=ALU.add
                        )
                        e2 = p_sm.tile([128, C * NBASIS], F32, tag="e2")
                        nc.scalar.activation(
                            e2[:jts], hgs[:jts, C * NBASIS :], AF.Exp, scale=-1.0
                        )
                        d2 = p_sm.tile([128, C * NBASIS], F32, tag="d2")
                        nc.vector.tensor_scalar_add(d2[:jts], e2[:jts], 1.0)
                        sg = p_sm.tile([128, C * NBASIS], F32, tag="sg")
                        scr2 = p_sm.tile([128, C * NBASIS], F32, tag="scr2")
                        nc.vector.reciprocal_approx_accurate(sg[:jts], d2[:jts], scr2[:jts])
                        hs = p_sm.tile([128, C * NBASIS], F32, tag="hs")
                        nc.vector.tensor_scalar(
                            hs[:jts], sg[:jts], 0.9, 0.1, op0=ALU.mult, op1=ALU.add
                        )
                        # z[j, kc*4+s] = mu[j,kc] + hs[j,kc] * eps[s,b,kc]
                        z = p_z.tile([128, NBASIS * C * NS], F32, tag="z")
                        zv = z[:jts].rearrange("p (kc s) -> p kc s", kc=NBASIS * C, s=NS)
                        hsv = hs[:jts].unsqueeze(2).broadcast_to([jts, NBASIS * C, NS])
                        ev = epss[b][:jts].rearrange(
                            "p (kc s) -> p kc s", kc=NBASIS * C, s=NS
                        )
                        nc.vector.tensor_tensor(zv, hsv, ev, op=ALU.mult)
                        muv = (
                            hgs[:jts, : C * NBASIS]
                            .unsqueeze(2)
                            .broadcast_to([jts, NBASIS * C, NS])
                        )
                        nc.vector.tensor_tensor(zv, zv, muv, op=ALU.add)
                        # zz2[j, (c,s,d)] = sum_k z[j,(k,c,s)] * kappa*loW[(k,c),d]
                        zzt = p_z.tile([128, C * NS * 2 * C * NBASIS], F32, tag="zzt")
                        zztv = zzt[:jts].rearrange(
                            "p (c s d k) -> p c s d k", c=C, s=NS, d=2 * C, k=NBASIS
                        )
                        zrv = (
                            z[:jts]
                            .rearrange("p (k c s) -> p c s k", k=NBASIS, c=C, s=NS)
                            .unsqueeze(3)
                            .broadcast_to([jts, C, NS, 2 * C, NBASIS])
                        )
                        lwv = lowb[:jts].rearrange(
                            "p (c s d k) -> p c s d k", c=C, s=NS, d=2 * C, k=NBASIS
                        )
                        nc.vector.tensor_tensor(zztv, zrv, lwv, op=ALU.mult)
                        zz2 = p_zz2.tile([128, C * NS * 2 * C], F32, tag="zz2")
                        nc.vector.reduce_sum(
                            zz2[:jts].rearrange("p (c s d) -> p c s d", c=C, s=NS, d=2 * C),
                            zztv,
                            axis=mybir.AxisListType.X,
                        )
                        zz2s.append(zz2)

                    # interp matmuls; both target tiles merged into one
                    # epilogue tile so the softplus ACTs run once per batch
                    ntt = NTAR // 128
                    w24 = NS * 2 * C
                    ot = p_ot.tile([128, ntt * w24], F32, tag="ot")
                    for tt in range(ntt):
                        po = ps_o.tile([128, w24], F32, tag="po")
                        nmm = 0
                        for jt in range(njt):
                            jts = mts[jt]
                            for c in range(C):
                                t0 = c * NTAR + tt * 128
                                nc.tensor.matmul(
                                    po[:],
                                    eis[b][jt][:jts, t0 : t0 + 128],
                                    zz2s[jt][:jts, c * w24 : (c + 1) * w24],
                                    start=(nmm == 0),
                                    stop=(nmm == njt * C - 1),
                                )
                                nmm += 1
                        nc.vector.tensor_tensor(
                            ot[:, tt * w24 : (tt + 1) * w24], po[:], lobb[:],
                            op=ALU.add,
                        )
                    # softplus on std cols (d in 3..5 per (tt,s) group)
                    sv = ot[:].rearrange(
                        "p (g d) -> p g d", g=ntt * NS, d=2 * C
                    )[:, :, C:]
                    av = p_sm.tile([128, ntt * NS * C], F32, tag="av")
                    avv = av[:].rearrange("p (g d) -> p g d", g=ntt * NS, d=C)
                    nc.scalar.activation(avv, sv, AF.Abs)
                    ew = p_sm.tile([128, ntt * NS * C], F32, tag="ew")
                    nc.scalar.activation(ew[:], av[:], AF.Exp, scale=-1.0)
                    lw_ = p_sm.tile([128, ntt * NS * C], F32, tag="lw_")
                    nc.scalar.activation(lw_[:], ew[:], AF.Ln, bias=1.0)
                    rv = p_sm.tile([128, ntt * NS * C], F32, tag="rv")
                    rvv = rv[:].rearrange("p (g d) -> p g d", g=ntt * NS, d=C)
                    nc.scalar.activation(rvv, sv, AF.Relu)
                    lvv = lw_[:].rearrange("p (g d) -> p g d", g=ntt * NS, d=C)
                    nc.vector.tensor_tensor(sv, rvv, lvv, op=ALU.add)
                    for tt in range(ntt):
                        for s in range(NS):
                            nc.sync.dma_start(
                                d_out.ap()[s, b, tt * 128 : (tt + 1) * 128, :],
                                ot[:, tt * w24 + s * 2 * C : tt * w24 + (s + 1) * 2 * C],
                            )

            # python-unrolled repeat for benchmarking (For_i's cross-iteration
            # sem reset wedged the device; straight-line repeats are safe)
            for _ in range(loop_r):
                body()

    nc.compile()
    return nc


def _prep(inputs):
    """Host-side slicing/replication. Returns (m, per-core in_maps)."""
    x = np.ascontiguousarray(inputs["x"], dtype=np.float32)
    y = np.ascontiguousarray(inputs["y"], dtype=np.float32)
    x_out = np.ascontiguousarray(inputs["x_out"], dtype=np.float32)
    x_grid = np.asarray(inputs["x_grid"], dtype=np.float32)
    eps_noise = np.asarray(inputs["eps_noise"], dtype=np.float32)
    enc_sigma = np.asarray(inputs["enc_sigma"], dtype=np.float64)
    int_sigma = np.asarray(inputs["int_sigma"], dtype=np.float64)
    gW = np.asarray(inputs["gW"], dtype=np.float32)
    gb = np.asarray(inputs["gb"], dtype=np.float32)
    w1 = np.asarray(inputs["w1"], dtype=np.float32)
    b1 = np.asarray(inputs["b1"], dtype=np.float32)
    w2 = np.asarray(inputs["w2"], dtype=np.float32)
    b2 = np.asarray(inputs["b2"], dtype=np.float32)
    w3 = np.asarray(inputs["w3"], dtype=np.float32)
    b3 = np.asarray(inputs["b3"], dtype=np.float32)
    linW = np.asarray(inputs["linW"], dtype=np.float32)
    linb = np.asarray(inputs["linb"], dtype=np.float32)
    loW = np.asarray(inputs["loW"], dtype=np.float32)
    lob = np.asarray(inputs["lob"], dtype=np.float32)

    nb, npts, _ = x.shape
    assert nb == NB and npts == NPTS
    m = x_grid.shape[1]
    g = x_grid[0, :, 0].astype(np.float32)

    # scales (match reference: 1/(exp(sigma)+EPS), folded with the 1/sqrt(2)
    # of exp(-0.5 d^2) = exp(-(d/sqrt2)^2))
    s_enc = np.exp(enc_sigma) + EPS           # (3,)
    alpha_enc = 1.0 / (np.sqrt(2.0) * s_enc)  # (3,)
    s_int = np.exp(int_sigma) + EPS           # (5,3)
    assert np.ptp(s_int) < 1e-12 * abs(s_int.flat[0]), "int_sigma must be uniform"
    alpha_int = float(1.0 / (np.sqrt(2.0) * s_int.flat[0]))
    _build.alpha_enc = [float(a) for a in alpha_enc]
    _build.alpha_int = alpha_int

    njt = (m + 127) // 128

    # xr: [b, p, ch*3+c] = x[b, ch*128+p, c]
    xr = x.reshape(NB, NCH, 128, C).transpose(0, 2, 1, 3).reshape(NB, 128, NCH * C)
    # yr: even cols 1.0 (scaled by kappa via gw instead -> keep 1.0/y raw)
    yr = np.empty((NB, 128, 2 * NCH * C), np.float32)
    yr[:, :, 0::2] = 1.0
    yr[:, :, 1::2] = xr * 0  # placeholder, filled below
    yrv = y.reshape(NB, NCH, 128, C).transpose(0, 2, 1, 3).reshape(NB, 128, NCH * C)
    yr[:, :, 1::2] = yrv
    # xtr: [b, p, c*256+t] = x_out[b,t,c] (replicated over p)
    xtr = np.broadcast_to(
        x_out.transpose(0, 2, 1).reshape(NB, 1, C * NTAR), (NB, 128, C * NTAR)
    ).copy()
    grep = np.broadcast_to(g[None, :], (128, m)).copy()
    # bj: [p, jt] = -g[jt*128+p] * alpha_int  (tail padded 0)
    gpad = np.zeros(njt * 128, np.float32)
    gpad[:m] = g
    bj = (-alpha_int * gpad).reshape(njt, 128).T.copy()
    # gw: rows 0-2 scaled by kappa (folds exp(-x^2) = kappa*DErf into h0)
    gw = gW.copy()
    gw[0:3] *= KAPPA
    gbn = (-gb).reshape(RIN, 1)
    # conv weights: wNt[ci, dk*32+o] = wN[o, ci, dk]
    w1t = w1.transpose(1, 2, 0).reshape(RIN, KW * ROUT).copy()
    w2t = w2.transpose(1, 2, 0).reshape(ROUT, KW * ROUT).copy()
    w3t = w3.transpose(1, 2, 0).reshape(ROUT, KW * ROUT).copy()
    b123 = np.stack([b1, b2, b3], axis=1)
    linbr = np.broadcast_to(linb[None, :], (128, 2 * C * NBASIS)).copy()
    # epsb: [b, p, kc*4+s] = eps_noise[s, b, kc]
    epsb = np.broadcast_to(
        eps_noise.transpose(1, 2, 0).reshape(NB, 1, NBASIS * C * NS),
        (NB, 128, NBASIS * C * NS),
    ).copy()
    # lowb: [p, ((c*4+s)*6+d)*5+k] = kappa * loW[k*3+c, d]
    lo = KAPPA * loW.reshape(NBASIS, C, 2 * C)
    lowb_vec = (
        np.broadcast_to(
            lo.transpose(1, 2, 0)[:, None, :, :], (C, NS, 2 * C, NBASIS)
        )
        .reshape(C * NS * 2 * C * NBASIS)
        .astype(np.float32)
    )
    lowb = np.broadcast_to(lowb_vec[None, :], (128, lowb_vec.size)).copy()
    # lobb: [p, s*6+d] = lob[d]
    lobb_vec = np.tile(lob, NS).astype(np.float32)
    lobb = np.broadcast_to(lobb_vec[None, :], (128, NS * 2 * C)).copy()
    alphas = np.zeros((1, 4), np.float32)

    in_maps = []
    for core in range(NCORES):
        bsl = slice(core * NBL, (core + 1) * NBL)
        in_maps.append(
            {
                "xr": xr[bsl].copy(),
                "yr": yr[bsl].copy(),
                "xtr": xtr[bsl].copy(),
                "grep": grep,
                "bj": bj,
                "gw": gw,
                "gbn": gbn,
                "w1t": w1t,
                "w2t": w2t,
                "w3t": w3t,
                "b123": b123,
                "linw": linW,
                "linbr": linbr,
                "epsb": epsb[bsl].copy(),
                "lowb": lowb,
                "lobb": lobb,
                "alphas": alphas,
            }
        )
    return m, in_maps


def kernel(**inputs):
    m, in_maps = _prep(inputs)
    key = ("k", m, _build.alpha_int, tuple(_build.alpha_enc))
    if key not in _CACHE:
        _CACHE[key] = _build(m, loop_r=1)
    nc = _CACHE[key]
    res = bass_utils.run_bass_kernel_spmd(nc, in_maps, core_ids=list(range(NCORES)))
    outs = [res.results[c]["out"] for c in range(NCORES)]  # each (NS, NBL, NTAR, 6)
    full = np.concatenate(outs, axis=1)  # (NS, NB, NTAR, 6)
    return full.astype(np.float32)



# revision 7
# speedup vs baseline: 2.2326x; 2.2326x over previous
"""Trainium2 Bass kernel for a latent ConvCNP (gaussian encoder -> CNN ->
latent samples -> gaussian interpolator), data-parallel over batch on 8
NeuronCores.

v2: sorted-point windowed encoder. Per (b,c) the 2048 context points are
host-sorted; each 128-point chunk then only overlaps a ~WF-column window of
the grid, at lattice positions W(ch) = (a - off) + q*ch (global affine fit).
The distance table shrinks from [128, 16*m] to [128, 16*WF] (~5x), computed
as one DVE subtract (overlapping-window AP on the grid tile) + one bf16
DErf ACT per (b,c); h0/h1 accumulate via start=False matmuls into an
eps-seeded psum. Normalization runs on [3, m] tiles after a partition
relocation DMA. Phase B uses native Sigmoid ACTs; softplus closes in a
third act-table phase.

Contract: kernel(**inputs) takes the full unsharded inputs (numpy) and
returns the full (NS, nb, ntar, 2C) output.
"""

import sys

sys.path.insert(0, "/opt/trn_rl_repo")

import math

import ml_dtypes
import numpy as np

import concourse.bacc as bacc
import concourse.mybir as mybir
import concourse.tile as tile
from concourse import bass_utils
from concourse.ap import AP

F32 = mybir.dt.float32
F32R = mybir.dt.float32r
BF16 = mybir.dt.bfloat16
AF = mybir.ActivationFunctionType
ALU = mybir.AluOpType

# problem constants (fixed by the reference problem)
EPS = 1e-6
C = 3
NBASIS = 5
NS = 4
RIN = 16
ROUT = 32
KW = 5
NB = 16          # full batch
NPTS = 2048
NTAR = 256
NCORES = 8
NBL = NB // NCORES   # batches per core
NCH = NPTS // 128    # 16 point-chunks per (b, c)
KAPPA = math.sqrt(math.pi) / 2.0  # exp(-x^2) = KAPPA * Derivative_Erf(x)
KREACH = 4.0                      # window reach in units of 1/alpha

_CACHE = {}


def _build(m, q, aoff, wf, mext, moff):
    """Per-core Bass program. m = grid cols (312); lattice W(ch) = aoff + q*ch
    (psum cols), window width wf, psum extent mext; grid col j lives at psum
    col j + moff (the gx input content is shifted to match)."""
    njt = (m + 127) // 128
    mts = [128] * (m // 128) + ([m % 128] if m % 128 else [])
    mp = m + 4  # padded conv width

    alpha_enc = _build.alpha_enc  # (3,) python floats
    alpha_int = _build.alpha_int  # float

    nc = bacc.Bacc("TRN2", target_bir_lowering=False, debug=False)

    # ---- per-core DRAM inputs ----
    d_xs = nc.dram_tensor("xs", [NBL, 128, NCH * C], F32, kind="ExternalInput")
    d_yi = nc.dram_tensor("yi", [NBL, 128, NCH * C * 2], BF16, kind="ExternalInput")
    d_xtr = nc.dram_tensor("xtr", [NBL, 128, C * NTAR], F32, kind="ExternalInput")
    d_gx = nc.dram_tensor("gx", [128, mext], F32, kind="ExternalInput")
    d_ion = nc.dram_tensor("ion", [1, 2 + mext], BF16, kind="ExternalInput")
    d_bj = nc.dram_tensor("bj", [128, njt], F32, kind="ExternalInput")
    d_gwab = nc.dram_tensor("gwab", [C, 2 * RIN], F32, kind="ExternalInput")
    d_gbn = nc.dram_tensor("gbn", [RIN, 1], F32, kind="ExternalInput")
    d_w1 = nc.dram_tensor("w1t", [RIN, KW * ROUT], F32, kind="ExternalInput")
    d_w2 = nc.dram_tensor("w2t", [ROUT, KW * ROUT], F32, kind="ExternalInput")
    d_w3 = nc.dram_tensor("w3t", [ROUT, KW * ROUT], F32, kind="ExternalInput")
    d_b123 = nc.dram_tensor("b123", [ROUT, 3], F32, kind="ExternalInput")
    d_linw = nc.dram_tensor("linw", [ROUT, 2 * C * NBASIS], F32, kind="ExternalInput")
    d_linbr = nc.dram_tensor("linbr", [128, 2 * C * NBASIS], F32, kind="ExternalInput")
    d_epsb = nc.dram_tensor("epsb", [NBL, 128, NBASIS * C * NS], F32, kind="ExternalInput")
    d_lowb = nc.dram_tensor("lowb", [128, C * NS * 2 * C * NBASIS], F32, kind="ExternalInput")
    d_lobb = nc.dram_tensor("lobb", [128, 2 * NS * 2 * C], F32, kind="ExternalInput")
    d_out = nc.dram_tensor("out", [NS, NBL, NTAR, 2 * C], F32, kind="ExternalOutput")

    with tile.TileContext(nc) as tc:
        import contextlib

        est = contextlib.ExitStack()
        with est:
            p_cst = est.enter_context(tc.tile_pool(name="cst", bufs=1))
            p_io = est.enter_context(tc.tile_pool(name="io", bufs=1))
            p_tab = est.enter_context(tc.tile_pool(name="tab", bufs=3))
            p_ei = est.enter_context(tc.tile_pool(name="ei", bufs=2 * njt))
            p_h = est.enter_context(tc.tile_pool(name="h", bufs=2))
            p_sm = est.enter_context(tc.tile_pool(name="sm", bufs=3))
            p_z = est.enter_context(tc.tile_pool(name="z", bufs=3))
            p_zz2 = est.enter_context(tc.tile_pool(name="zz2", bufs=njt + 1))
            p_ot = est.enter_context(tc.tile_pool(name="ot", bufs=2))
            ps_e = est.enter_context(tc.tile_pool(name="pse", bufs=4, space="PSUM"))
            ps_r = est.enter_context(tc.tile_pool(name="psr", bufs=1, space="PSUM"))
            ps_c = est.enter_context(tc.tile_pool(name="psc", bufs=1, space="PSUM"))
            ps_h = est.enter_context(tc.tile_pool(name="psh", bufs=1, space="PSUM"))
            ps_o = est.enter_context(tc.tile_pool(name="pso", bufs=1, space="PSUM"))

            # ---- input DMAs (xtr/bj first: the eis ACTs are queue-head) ----
            xtrs, xss, yis, epss = [], [], [], []
            for b in range(NBL):
                xtr = p_io.tile([128, C * NTAR], F32, tag=f"xtr{b}")
                nc.sync.dma_start(xtr[:], d_xtr.ap()[b])
                xtrs.append(xtr)
            bj = p_cst.tile([128, njt], F32)
            nc.sync.dma_start(bj[:], d_bj.ap())
            gx = p_cst.tile([128, mext], F32)
            nc.sync.dma_start(gx[:], d_gx.ap())
            for b in range(NBL):
                xs = p_io.tile([128, NCH * C], F32, tag=f"xs{b}")
                yi = p_io.tile([128, NCH * C * 2], BF16, tag=f"yi{b}")
                nc.sync.dma_start(xs[:], d_xs.ap()[b])
                nc.sync.dma_start(yi[:], d_yi.ap()[b])
                xss.append(xs)
                yis.append(yi)
            ion = p_cst.tile([1, 2 + mext], BF16)
            nc.sync.dma_start(ion[:], d_ion.ap())
            gwab = p_cst.tile([C, 2 * RIN], F32R)
            gbn = p_cst.tile([RIN, 1], F32)
            w1 = p_cst.tile([RIN, KW * ROUT], F32R)
            w2 = p_cst.tile([ROUT, KW * ROUT], F32R)
            w3 = p_cst.tile([ROUT, KW * ROUT], F32R)
            b123 = p_cst.tile([ROUT, 3], F32)
            linw = p_cst.tile([ROUT, 2 * C * NBASIS], F32R)
            linbr = p_cst.tile([128, 2 * C * NBASIS], F32)
            nc.sync.dma_start(gwab[:], d_gwab.ap().bitcast(F32R))
            nc.sync.dma_start(gbn[:], d_gbn.ap())
            nc.sync.dma_start(w1[:], d_w1.ap().bitcast(F32R))
            nc.sync.dma_start(w2[:], d_w2.ap().bitcast(F32R))
            nc.sync.dma_start(w3[:], d_w3.ap().bitcast(F32R))
            nc.sync.dma_start(b123[:], d_b123.ap())
            nc.sync.dma_start(linw[:], d_linw.ap().bitcast(F32R))
            nc.sync.dma_start(linbr[:], d_linbr.ap())
            epsbs, lowb, lobb = [], None, None
            for b in range(NBL):
                epsb = p_io.tile([128, NBASIS * C * NS], F32, tag=f"eps{b}")
                nc.sync.dma_start(epsb[:], d_epsb.ap()[b])
                epsbs.append(epsb)
            lowb = p_cst.tile([128, C * NS * 2 * C * NBASIS], F32)
            lobb = p_cst.tile([128, 2 * NS * 2 * C], F32)
            nc.sync.dma_start(lowb[:], d_lowb.ap())
            nc.sync.dma_start(lobb[:], d_lobb.ap())

            # conv act tiles (dedicated; pads zeroed once on Pool)
            h0cs = [
                p_cst.tile([RIN, mp], F32R, name=f"h0c{i}") for i in range(NBL)
            ]
            h1cs = [
                p_cst.tile([ROUT, mp], F32R, name=f"h1c{i}") for i in range(NBL)
            ]
            h2cs = [
                p_cst.tile([ROUT, mp], F32R, name=f"h2c{i}") for i in range(NBL)
            ]
            for t in h0cs + h1cs + h2cs:
                nc.gpsimd.memset(t[:, 0:2].bitcast(F32), 0.0)
                nc.gpsimd.memset(t[:, 2 + m : mp].bitcast(F32), 0.0)

            # ---- interp gaussian tables (DErf, ACT queue head) ----
            eis = [[], []]
            for b in range(NBL):
                for jt in range(njt):
                    jts = mts[jt]
                    ei = p_ei.tile([128, C * NTAR], F32, tag="ei")
                    nc.scalar.activation(
                        ei[:jts], xtrs[b][:jts], AF.Derivative_Erf,
                        bias=bj[:jts, jt : jt + 1], scale=float(alpha_int),
                    )
                    eis[b].append(ei)

            # ---- encoder tables + accumulation ----
            gap = gx[:]
            win = AP(gap.tensor, gap.offset + aoff,
                     [list(gap.ap[0]), [q, NCH], [1, wf]])
            psums = [[None] * C for _ in range(NBL)]
            for b in range(NBL):
                for c in range(C):
                    d6 = p_tab.tile([128, NCH * wf], F32, tag="d6")
                    xv = (
                        xss[b][:]
                        .rearrange("p (ch c) -> p ch c", ch=NCH, c=C)[:, :, c]
                        .unsqueeze(2)
                        .broadcast_to([128, NCH, wf])
                    )
                    d6v = d6[:].rearrange("p (ch r) -> p ch r", ch=NCH, r=wf)
                    nc.vector.tensor_tensor(d6v, win, xv, op=ALU.subtract)
                    e6 = p_tab.tile([128, NCH * wf], BF16, tag="e6")
                    nc.scalar.activation(
                        e6[:], d6[:], AF.Derivative_Erf, scale=float(alpha_enc[c])
                    )
                    ps2 = ps_e.tile([2, mext], F32, tag="pse")
                    nc.tensor.matmul(
                        ps2[:], ion[0:1, 0:2], ion[0:1, 2:],
                        start=True, stop=False, skip_group_check=True,
                    )
                    for ch in range(NCH):
                        s0 = aoff + q * ch
                        nc.tensor.matmul(
                            ps2[:, s0 : s0 + wf],
                            yis[b][:, (ch * C + c) * 2 : (ch * C + c) * 2 + 2],
                            e6[:, ch * wf : (ch + 1) * wf],
                            start=False, stop=(ch == NCH - 1),
                            skip_group_check=True,
                        )
                    psums[b][c] = ps2

            # ---- psum -> staging, relocation, bridge ----
            fH0s, nh3s = [], []
            for b in range(NBL):
                hcat = p_h.tile([2, C * m], F32R, tag="hcat")
                for c in range(C):
                    nc.vector.tensor_copy(
                        hcat[:, c * m : (c + 1) * m].bitcast(F32),
                        psums[b][c][:, moff : moff + m],
                    )
                fH0 = p_h.tile([C, m], F32R, tag="fH0")
                fH1 = p_h.tile([C, m], F32R, tag="fH1")
                nc.sync.dma_start(
                    fH0[:], hcat[0:1].rearrange("one (c m) -> one c m", c=C, m=m)
                )
                nc.sync.dma_start(
                    fH1[:], hcat[1:2].rearrange("one (c m) -> one c m", c=C, m=m)
                )
                rec3 = p_h.tile([C, m], F32, tag="rec3")
                nc.vector.reciprocal_approx_fast(rec3[:], fH0[:].bitcast(F32))
                nh3 = p_h.tile([C, m], F32R, tag="nh3")
                nc.vector.tensor_tensor(
                    nh3[:], fH1[:].bitcast(F32), rec3[:], op=ALU.mult
                )
                fH0s.append(fH0)
                nh3s.append(nh3)

            # ---- phase B (sigmoid table): rho CNN + latent z + interp ----
            ots = []
            for b in range(NBL):
                rp = ps_r.tile([RIN, m], F32, tag="rp")
                nc.tensor.matmul(rp[:], gwab[:, :RIN], fH0s[b][:],
                                 start=True, stop=False, skip_group_check=True)
                nc.tensor.matmul(rp[:], gwab[:, RIN:], nh3s[b][:],
                                 start=False, stop=True, skip_group_check=True)
                h0c = h0cs[b]
                nc.scalar.activation(
                    h0c[:, 2 : 2 + m], rp[:], AF.Sigmoid, bias=gbn[:], scale=1.0
                )
                hin = h0c
                for li, (wt, cin, hnext) in enumerate(
                    [(w1, RIN, h1cs[b]), (w2, ROUT, h2cs[b]), (w3, ROUT, None)]
                ):
                    cps = ps_c.tile([ROUT, m], F32, tag="cps")
                    for dk in range(KW):
                        nc.tensor.matmul(
                            cps[:], wt[:cin, dk * ROUT : (dk + 1) * ROUT],
                            hin[:cin, dk : dk + m],
                            start=(dk == 0), stop=(dk == KW - 1),
                        )
                    if li < 2:
                        nc.scalar.activation(
                            hnext[:, 2 : 2 + m], cps[:], AF.Relu,
                            bias=b123[:, li : li + 1], scale=1.0,
                        )
                        hin = hnext
                    else:
                        h3 = p_h.tile([ROUT, m], F32R, tag="h3")
                        nc.scalar.activation(
                            h3[:], cps[:], AF.Identity, bias=b123[:, 2:3], scale=1.0
                        )

                # h_grid -> z -> zz2 per grid tile
                zz2s = []
                for jt in range(njt):
                    jts = mts[jt]
                    j0 = jt * 128
                    hg = ps_h.tile([128, 2 * C * NBASIS], F32, tag="hg")
                    nc.tensor.matmul(
                        hg[:jts], h3[:, j0 : j0 + jts], linw[:], start=True, stop=True
                    )
                    hgs = p_sm.tile([128, 2 * C * NBASIS], F32, tag="hgs")
                    nc.vector.tensor_tensor(
                        hgs[:jts], hg[:jts], linbr[:jts], op=ALU.add
                    )
                    sg = p_sm.tile([128, C * NBASIS], F32, tag="sg")
                    nc.scalar.activation(
                        sg[:jts], hgs[:jts, C * NBASIS :], AF.Sigmoid
                    )
                    hs = p_sm.tile([128, C * NBASIS], F32, tag="hs")
                    nc.vector.tensor_scalar(
                        hs[:jts], sg[:jts], 0.9, 0.1, op0=ALU.mult, op1=ALU.add
                    )
                    z = p_z.tile([128, NBASIS * C * NS], F32, tag="z")
                    zv = z[:jts].rearrange("p (kc s) -> p kc s", kc=NBASIS * C, s=NS)
                    hsv = hs[:jts].unsqueeze(2).broadcast_to([jts, NBASIS * C, NS])
                    ev = epsbs[b][:jts].rearrange(
                        "p (kc s) -> p kc s", kc=NBASIS * C, s=NS
                    )
                    nc.vector.tensor_tensor(zv, hsv, ev, op=ALU.mult)
                    muv = (
                        hgs[:jts, : C * NBASIS]
                        .unsqueeze(2)
                        .broadcast_to([jts, NBASIS * C, NS])
                    )
                    nc.vector.tensor_tensor(zv, zv, muv, op=ALU.add)
                    zzt = p_z.tile([128, C * NS * 2 * C * NBASIS], F32, tag="zzt")
                    zztv = zzt[:jts].rearrange(
                        "p (c s d k) -> p c s d k", c=C, s=NS, d=2 * C, k=NBASIS
                    )
                    zrv = (
                        z[:jts]
                        .rearrange("p (k c s) -> p c s k", k=NBASIS, c=C, s=NS)
                        .unsqueeze(3)
                        .broadcast_to([jts, C, NS, 2 * C, NBASIS])
                    )
                    lwv = lowb[:jts].rearrange(
                        "p (c s d k) -> p c s d k", c=C, s=NS, d=2 * C, k=NBASIS
                    )
                    nc.vector.tensor_tensor(zztv, zrv, lwv, op=ALU.mult)
                    zz2 = p_zz2.tile([128, C * NS * 2 * C], F32, tag="zz2")
                    nc.vector.reduce_sum(
                        zz2[:jts].rearrange(
                            "p (c s d) -> p c s d", c=C, s=NS, d=2 * C
                        ),
                        zztv,
                        axis=mybir.AxisListType.X,
                    )
                    zz2s.append(zz2)

                # interp matmuls
                ntt = NTAR // 128
                w24 = NS * 2 * C
                ot = p_ot.tile([128, ntt * w24], F32, tag="ot")
                for tt in range(ntt):
                    po = ps_o.tile([128, w24], F32, tag="po")
                    nmm = 0
                    for jt in range(njt):
                        jts = mts[jt]
                        for c in range(C):
                            t0 = c * NTAR + tt * 128
                            nc.tensor.matmul(
                                po[:],
                                eis[b][jt][:jts, t0 : t0 + 128],
                                zz2s[jt][:jts, c * w24 : (c + 1) * w24],
                                start=(nmm == 0), stop=(nmm == njt * C - 1),
                            )
                            nmm += 1
                    nc.vector.tensor_tensor(
                        ot[:, tt * w24 : (tt + 1) * w24], po[:], lobb[:, tt * w24 : (tt + 1) * w24],
                        op=ALU.add,
                    )
                ots.append(ot)

            # ---- phase C (ln/exp table): softplus on std cols, then out ----
            ntt = NTAR // 128
            w24 = NS * 2 * C
            for b in range(NBL):
                ot = ots[b]
                sv = ot[:].rearrange(
                    "p (g d) -> p g d", g=ntt * NS, d=2 * C
                )[:, :, C:]
                av = p_sm.tile([128, ntt * NS * C], F32, tag="av")
                avv = av[:].rearrange("p (g d) -> p g d", g=ntt * NS, d=C)
                nc.scalar.activation(avv, sv, AF.Abs)
                ew = p_sm.tile([128, ntt * NS * C], F32, tag="ew")
                nc.scalar.activation(ew[:], av[:], AF.Exp, scale=-1.0)
                lw_ = p_sm.tile([128, ntt * NS * C], F32, tag="lw_")
                nc.scalar.activation(lw_[:], ew[:], AF.Ln, bias=1.0)
                rv = p_sm.tile([128, ntt * NS * C], F32, tag="rv")
                rvv = rv[:].rearrange("p (g d) -> p g d", g=ntt * NS, d=C)
                nc.scalar.activation(rvv, sv, AF.Relu)
                lvv = lw_[:].rearrange("p (g d) -> p g d", g=ntt * NS, d=C)
                nc.vector.tensor_tensor(sv, rvv, lvv, op=ALU.add)
                for tt in range(ntt):
                    nc.sync.dma_start(
                        d_out.ap()[:, b, tt * 128 : (tt + 1) * 128, :].rearrange(
                            "s p d -> p s d"
                        ),
                        ot[:, tt * w24 : (tt + 1) * w24].rearrange(
                            "p (s d) -> p s d", s=NS, d=2 * C
                        ),
                    )

    nc.compile()
    return nc


def _prep(inputs):
    """Host-side sorting/slicing/replication. Returns (key, per-core in_maps)."""
    x = np.ascontiguousarray(inputs["x"], dtype=np.float32)
    y = np.ascontiguousarray(inputs["y"], dtype=np.float32)
    x_out = np.ascontiguousarray(inputs["x_out"], dtype=np.float32)
    x_grid = np.asarray(inputs["x_grid"], dtype=np.float32)
    eps_noise = np.asarray(inputs["eps_noise"], dtype=np.float32)
    enc_sigma = np.asarray(inputs["enc_sigma"], dtype=np.float64)
    int_sigma = np.asarray(inputs["int_sigma"], dtype=np.float64)
    gW = np.asarray(inputs["gW"], dtype=np.float32)
    gb = np.asarray(inputs["gb"], dtype=np.float32)
    w1 = np.asarray(inputs["w1"], dtype=np.float32)
    b1 = np.asarray(inputs["b1"], dtype=np.float32)
    w2 = np.asarray(inputs["w2"], dtype=np.float32)
    b2 = np.asarray(inputs["b2"], dtype=np.float32)
    w3 = np.asarray(inputs["w3"], dtype=np.float32)
    b3 = np.asarray(inputs["b3"], dtype=np.float32)
    linW = np.asarray(inputs["linW"], dtype=np.float32)
    linb = np.asarray(inputs["linb"], dtype=np.float32)
    loW = np.asarray(inputs["loW"], dtype=np.float32)
    lob = np.asarray(inputs["lob"], dtype=np.float32)

    nb, npts, _ = x.shape
    assert nb == NB and npts == NPTS
    m = x_grid.shape[1]
    g = x_grid[0, :, 0].astype(np.float64)
    g0 = float(g[0])
    gd = float((g[-1] - g[0]) / (m - 1))

    s_enc = np.exp(enc_sigma) + EPS           # (3,)
    alpha_enc = 1.0 / (np.sqrt(2.0) * s_enc)  # (3,)
    s_int = np.exp(int_sigma) + EPS           # (5,3)
    assert np.ptp(s_int) < 1e-12 * abs(s_int.flat[0]), "int_sigma must be uniform"
    alpha_int = float(1.0 / (np.sqrt(2.0) * s_int.flat[0]))
    _build.alpha_enc = [float(a) for a in alpha_enc]
    _build.alpha_int = alpha_int

    njt = (m + 127) // 128

    # ---- sort points per (b, c); global affine window lattice ----
    xs_all = np.empty((NB, C, NPTS), np.float32)
    ys_all = np.empty((NB, C, NPTS), np.float32)
    for b in range(NB):
        for c in range(C):
            idx = np.argsort(x[b, :, c], kind="stable")
            xs_all[b, c] = x[b, idx, c]
            ys_all[b, c] = y[b, idx, c]
    chunks = xs_all.reshape(NB, C, NCH, 128)
    reach = KREACH / alpha_enc.reshape(1, 3, 1)               # (1,3,1) x-units
    c_lo = np.ceil((chunks[:, :, :, 0] - reach - g0) / gd).astype(int)    # (NB,C,NCH)
    c_hi = np.floor((chunks[:, :, :, -1] + reach - g0) / gd).astype(int)
    ch_idx = np.arange(NCH)
    qfit = (c_lo[:, :, -1] + c_hi[:, :, -1] - c_lo[:, :, 0] - c_hi[:, :, 0]) / (
        2.0 * (NCH - 1)
    )
    q = int(round(float(np.median(qfit))))
    a = int((c_lo - q * ch_idx).min())
    whi = int((c_hi - q * ch_idx).max())
    wf = whi - a + 1
    off = min(a, 0)
    aoff = a - off                       # psum col of window ch=0
    mext = max(m, a + q * (NCH - 1) + wf) - off
    assert mext <= 512, f"psum extent {mext} > 512"
    assert wf <= 128, f"window {wf} too wide"

    # ---- device input tensors ----
    xsr = xs_all.reshape(NB, C, NCH, 128).transpose(0, 3, 2, 1).reshape(
        NB, 128, NCH * C
    )  # [b, p, ch*C+c]
    yi = np.empty((NB, 128, NCH * C * 2), np.float32)
    yi[:, :, 0::2] = 1.0
    yi[:, :, 1::2] = ys_all.reshape(NB, C, NCH, 128).transpose(0, 3, 2, 1).reshape(
        NB, 128, NCH * C
    )
    yi = yi.astype(ml_dtypes.bfloat16)
    xtr = np.broadcast_to(
        x_out.transpose(0, 2, 1).reshape(NB, 1, C * NTAR), (NB, 128, C * NTAR)
    ).copy()
    gxv = (g0 + gd * (np.arange(mext) + off)).astype(np.float32)
    gx = np.broadcast_to(gxv, (128, mext)).copy()
    ion = np.zeros((1, 2 + mext), np.float32)
    ion[0, 0] = EPS / KAPPA
    ion[0, 2:] = 1.0
    ion = ion.astype(ml_dtypes.bfloat16)
    gpad = np.zeros(njt * 128, np.float32)
    gpad[:m] = g.astype(np.float32)
    bj = (-alpha_int * gpad).reshape(njt, 128).T.copy()
    gwab = np.concatenate([KAPPA * gW[0:3], gW[3:6]], axis=1)  # [3, 2*RIN]
    gbn = gb.reshape(RIN, 1).copy()
    w1t = w1.transpose(1, 2, 0).reshape(RIN, KW * ROUT).copy()
    w2t = w2.transpose(1, 2, 0).reshape(ROUT, KW * ROUT).copy()
    w3t = w3.transpose(1, 2, 0).reshape(ROUT, KW * ROUT).copy()
    b123 = np.stack([b1, b2, b3], axis=1)
    linbr = np.broadcast_to(linb[None, :], (128, 2 * C * NBASIS)).copy()
    epsb = np.broadcast_to(
        eps_noise.transpose(1, 2, 0).reshape(NB, 1, NBASIS * C * NS),
        (NB, 128, NBASIS * C * NS),
    ).copy()
    lo = KAPPA * loW.reshape(NBASIS, C, 2 * C)
    lowb_vec = (
        np.broadcast_to(lo.transpose(1, 2, 0)[:, None, :, :], (C, NS, 2 * C, NBASIS))
        .reshape(C * NS * 2 * C * NBASIS)
        .astype(np.float32)
    )
    lowb = np.broadcast_to(lowb_vec[None, :], (128, lowb_vec.size)).copy()
    lobb_vec = np.tile(lob, 2 * NS).astype(np.float32)   # (tt, s, d)
    lobb = np.broadcast_to(lobb_vec[None, :], (128, 2 * NS * 2 * C)).copy()

    in_maps = []
    for core in range(NCORES):
        bsl = slice(core * NBL, (core + 1) * NBL)
        in_maps.append(
            {
                "xs": xsr[bsl].copy(),
                "yi": np.ascontiguousarray(yi[bsl]),
                "xtr": xtr[bsl].copy(),
                "gx": gx,
                "ion": ion,
                "bj": bj,
                "gwab": gwab,
                "gbn": gbn,
                "w1t": w1t,
                "w2t": w2t,
                "w3t": w3t,
                "b123": b123,
                "linw": linW,
                "linbr": linbr,
                "epsb": epsb[bsl].copy(),
                "lowb": lowb,
                "lobb": lobb,
            }
        )
    key = (m, q, aoff, wf, mext, -off, _build.alpha_int, tuple(_build.alpha_enc))
    return key, in_maps


def kernel(**inputs):
    key, in_maps = _prep(inputs)
    if key not in _CACHE:
        _CACHE[key] = _build(*key[:6])
    nc = _CACHE[key]
    res = bass_utils.run_bass_kernel_spmd(nc, in_maps, core_ids=list(range(NCORES)))
    outs = [res.results[c]["out"] for c in range(NCORES)]  # each (NS, NBL, NTAR, 6)
    full = np.concatenate(outs, axis=1)  # (NS, NB, NTAR, 6)
    return full.astype(np.float32)
